# revision 37
# baseline (speedup 1.0000x reference)
"""Trainium2 Bass kernel for nn_BCTransformer: B=131072 batch of tiny 2-token
4-layer transformer encoder forward passes.

Pure data parallel over 8 NeuronCores (16384 batch each).  Feature-major
layout: [D=128 partitions, columns], columns = (token, batch); super-tiles of
1024 columns (512 batch x 2 tokens); 32 super-tiles per core.

Centered-residual-stream design:
 - The residual state kept is hc = C@h (C = I - J/128, idempotent).  C is
   folded host-side into every weight that writes the stream, so LayerNorm
   needs NO centering matmul: var comes straight from Square(hc+cb) + an
   all-1/256 matmul.
 - hc lives in PSUM for the whole tile; residual adds are matmul
   accumulations (start=False) into the same banks.  Additive biases never
   enter hc: they are applied at each LN read point, riding the ACT Square
   bias and a fused (hc+cb)*R scalar_tensor_tensor.
 - rstd: magic-constant seed (DVE shift/xor + Pool int add) + one custom DVE
   op doing 2 Newton iterations.  eps is accumulated via an init-matmul on
   the embed LN only (elsewhere var >> eps so it is numerically irrelevant).
 - emission is phase-major across ILV in-flight super-tiles, with ONE psum
   transient bank + ONE hc bank per tile (explicit slot reuse) and per-tile
   SBUF scratch sets, so tiles genuinely pipeline; HW is instruction-rate
   limited (~1.5us/instr), so ops are kept maximally wide ([128,1024]).
 - attention: softmax over S=2 == 0.5+0.5*tanh(d/2).  dk/dv are computed as
   W@(y0-y1) (k/v biases cancel); the (v0+v1) path is folded host-side into
   0.5*C@out_w@Wv so k/v are never materialized.  Score path runs in bf16.
"""
import sys

sys.path.insert(0, "/opt/trn_rl_repo")

import math
from contextlib import ExitStack

import numpy as np

import concourse.bass as bass
import concourse.tile as tile
from concourse import bacc, mybir
from concourse.bass_utils import run_bass_kernel_spmd

# ---------------------------------------------------------------- constants
D = 128
NH = 4
HD = 32
FF = 256
L = 4
S = 2
B = 131072
EPS = 1e-5
NCORES = 8
BP = B // NCORES          # batch per core = 16384
N = 512                   # batch elems per super-tile
NT = 2 * N                # columns per super-tile (tok0 | tok1)
NTILES = BP // N          # super-tiles per core
ILV = 2                   # super-tiles in flight
MMC = 512                 # matmul free-dim chunk (PSUM bank limit)

F32 = mybir.dt.float32
F32R = mybir.dt.float32r
BF16 = mybir.dt.bfloat16
I32 = mybir.dt.int32
AF = mybir.ActivationFunctionType
ALU = mybir.AluOpType

# ------------------------------------------------- custom DVE rsqrt op
MAGIC = 0x5F375A86
MAGIC_VH = MAGIC - (1 << 22)   # seed computed from bits of vh = v/2
SEED_ADD = MAGIC_VH + 1        # seed = ~(i_vh >> 1) + SEED_ADD


def _register_rsqrt_op():
    """y = NR2(seed, vh) ~= 1/sqrt(2*vh); C0=1.5."""
    import concourse.dve_ops as dve_ops
    from concourse.dve_ops import DveOp
    from concourse.dve_spec import C0, Spec, Src0, Src1, lower, _has_src1
    from concourse.dve_uop import DveOpSpec

    name = "RSQRT_NR2_ANT"
    if name in dve_ops._SUB_OPCODE_FOR_NAME:
        for op in dve_ops.OPS:
            if op.name == name:
                return op

    def _ref(in0, in1, c0, c1, c2):
        vh = in0.astype(np.float32)
        s = in1.astype(np.float32)
        y1 = s * (c0 - vh * s * s)
        y2 = y1 * (c0 - vh * y1 * y1)
        return y2.astype(np.float32)

    _y1 = Src1 * (C0 - Src0 * (Src1 * Src1))
    spec = Spec(body=_y1 * (C0 - Src0 * (_y1 * _y1)), reference=_ref)
    opcode = dve_ops._CUSTOM_DVE_ROW_BASE + len(dve_ops.OPS)
    assert opcode < 0x20
    dve_ops._SUB_OPCODE_FOR_NAME[name] = opcode
    shas = {}
    for ver in ("v3", "v4"):
        try:
            uops = lower(spec, ver=ver)
            shas[ver] = DveOpSpec(
                name=name, opcode=opcode, uops=uops, rd1_en=_has_src1(spec)
            ).sha(ver)
        except Exception:
            pass
    op = DveOp(name, spec, subdim=False, uops_sha=shas)
    dve_ops.OPS.append(op)
    dve_ops.CUSTOM_DVE_SPECS[name] = spec
    return op


RSQRT_NR2 = _register_rsqrt_op()


# ------------------------------------------------- host-side weight folding
def _prepare_weights(p):
    f = lambda a: np.asarray(a, np.float64)
    C = np.eye(D) - 1.0 / D                      # centering projector

    out = {}
    # embed: ec = (C @ W_in sqrt(D)) @ t ; read-bias cbin = C @ (b_in sqrt(D))
    w_in = f(p["w_in"]) * math.sqrt(D)
    out["wx"] = np.ascontiguousarray((C @ w_in).T).astype(np.float32)  # [2,128]
    cbin = C @ (f(p["b_in"]) * math.sqrt(D))

    # positional encoding
    pos = np.arange(10, dtype=np.float64)[:, None]
    div = np.exp(np.arange(0, D, 2, dtype=np.float64) * (-math.log(10000.0) / D))
    pe = np.zeros((10, D))
    pe[:, 0::2] = np.sin(pos * div)
    pe[:, 1::2] = np.cos(pos * div)

    # hc0 = Cg @ y_e  (+ bias cpe_tok at later reads)
    Cg = C @ np.diag(f(p["g_in"]))
    out["wcg"] = np.ascontiguousarray(Cg.T).astype(np.float32)  # [128,128]
    bias0 = C @ (f(p["bt_in"]) + pe[0])
    bias1 = C @ (f(p["bt_in"]) + pe[1])

    ln_bias = [(cbin.copy(), cbin.copy())]       # embed-LN read point
    q_bias, g_bias = [], []
    wl = np.zeros((L, 128, 9 * 128), np.float32)
    for l in range(L):
        ln_bias.append((bias0.copy(), bias1.copy()))   # LN1 of layer l
        g1 = f(p["n1_g"][l]); b1 = f(p["n1_b"][l])
        qkv_w = f(p["qkv_w"][l]); qkv_b = f(p["qkv_b"][l])
        Wq, Wk, Wv = qkv_w[0:128], qkv_w[128:256], qkv_w[256:384]
        Wqg, Wkg, Wvg = Wq * g1, Wk * g1, Wv * g1
        bq = qkv_b[0:128] + Wq @ b1
        bv = qkv_b[256:384] + Wv @ b1
        out_w = f(p["out_w"][l])
        CWV = 0.5 * (C @ out_w @ Wvg)            # sy path
        CWh = 0.5 * (C @ out_w)                  # u path
        g2 = f(p["n2_g"][l]); b2 = f(p["n2_b"][l])
        ff1_w = f(p["ff1_w"][l]); ff1_b = f(p["ff1_b"][l])
        F1g = ff1_w * g2
        bf = ff1_b + ff1_w @ b2
        Mf2 = C @ f(p["ff2_w"][l])               # [128, 256]

        wl[l, :, 0 * 128:1 * 128] = Wqg.T
        wl[l, :, 1 * 128:2 * 128] = Wkg.T
        wl[l, :, 2 * 128:3 * 128] = Wvg.T
        wl[l, :, 3 * 128:4 * 128] = CWV.T
        wl[l, :, 4 * 128:5 * 128] = CWh.T
        wl[l, :, 5 * 128:6 * 128] = F1g[0:128].T
        wl[l, :, 6 * 128:7 * 128] = F1g[128:256].T
        wl[l, :, 7 * 128:8 * 128] = Mf2[:, 0:128].T
        wl[l, :, 8 * 128:9 * 128] = Mf2[:, 128:256].T
        q_bias.append(bq)
        g_bias.append((bf[0:128], bf[128:256]))

        # residual-stream bias accumulation (never added to hc itself)
        delta = C @ (f(p["out_b"][l]) + out_w @ bv)
        bias0 += delta; bias1 += delta
        ln_bias.append((bias0.copy(), bias1.copy()))   # LN2 of layer l
        delta2 = C @ f(p["ff2_b"][l])
        bias0 += delta2; bias1 += delta2
    ln_bias.append((bias0.copy(), bias1.copy()))       # final-LN read point
    out["wl"] = wl

    # head: p = 0.5*(y0+y1); h1 folds g_out and the 0.5
    go = f(p["g_out"]); bo = f(p["bt_out"])
    h1_w = f(p["h1_w"])
    wh = np.zeros((128, 193), np.float32)
    wh[:, 0:128] = (0.5 * h1_w * go).T
    wh[:, 128:192] = f(p["h2_w"]).T
    wh[0:64, 192] = f(p["h3_w"])[0]
    out["wh"] = wh
    bh1 = f(p["h1_b"]) + h1_w @ bo

    # bias tile: q (4), gelu (8), head (3)
    nb = 22 + 4 + 8 + 3
    bias = np.zeros((128, nb), np.float64)
    for l in range(L):
        bias[:, 22 + l] = q_bias[l]
        bias[:, 26 + 2 * l] = g_bias[l][0]
        bias[:, 27 + 2 * l] = g_bias[l][1]
    bias[:, 34] = bh1
    bias[0:64, 35] = f(p["h2_b"])
    bias[0, 36] = f(p["h3_b"])[0]
    out["bias"] = bias.astype(np.float32)
    # bias DELTAS accumulated straight into the psum residual stream
    # (all C-projected => mean-zero => the centered invariant holds):
    # row 0 cbin (into ec), 1 cpe0, 2 cpe1 (into hc init),
    # 3+2l attn delta, 4+2l ff delta
    bd = np.zeros((11, 128), np.float64)
    bd[0] = ln_bias[0][0]
    bd[1] = ln_bias[1][0]
    bd[2] = ln_bias[1][1]
    for l in range(L):
        bd[3 + 2 * l] = ln_bias[2 + 2 * l][0] - ln_bias[1 + 2 * l][0]
        if l < L - 1:
            bd[4 + 2 * l] = ln_bias[3 + 2 * l][0] - ln_bias[2 + 2 * l][0]
        else:
            bd[4 + 2 * l] = ln_bias[9][0] - ln_bias[2 + 2 * l][0]
    out["biasd"] = bd.astype(np.float32)
    return out


def _static_consts():
    c = {}
    c["Jv"] = np.full((128, 128), 1.0 / 256.0, np.float32)
    sm = np.zeros((128, 4), np.float32)
    for d in range(128):
        sm[d, d // HD] = 1.0 / math.sqrt(HD)
    c["smask"] = sm
    bc = np.zeros((36, 256), np.float32)
    for d in range(128):
        bc[0 + d // HD, 0 * 128 + d] = 1.0
        bc[32 + d // HD, 1 * 128 + d] = 1.0
    c["bcmask"] = bc
    return c


def r32(ap):
    return ap.bitcast(F32R)


def build_nc(ntiles=NTILES, bd_zero=(False,) * 11):
    nc = bacc.Bacc(None, target_bir_lowering=False)
    cst = _static_consts()

    x_d = nc.dram_tensor("x", [BP, 4], F32, kind="ExternalInput")
    wx_d = nc.dram_tensor("wx", [2, 128], F32, kind="ExternalInput")
    wcg_d = nc.dram_tensor("wcg", [128, 128], F32, kind="ExternalInput")
    wl_d = nc.dram_tensor("wl", [L, 128, 9 * 128], F32, kind="ExternalInput")
    wh_d = nc.dram_tensor("wh", [128, 193], F32, kind="ExternalInput")
    bias_d = nc.dram_tensor("bias", [128, 37], F32, kind="ExternalInput")
    biasd_d = nc.dram_tensor("biasd", [11, 128], F32, kind="ExternalInput")
    o_d = nc.dram_tensor("o", [1, BP], F32, kind="ExternalOutput")

    Jv_d = nc.inline_tensor(cst["Jv"], name="Jvmat")
    sm_d = nc.inline_tensor(cst["smask"], name="smask")
    bc_d = nc.inline_tensor(cst["bcmask"], name="bcmask")

    with tile.TileContext(nc) as tc, ExitStack() as ctx:
        wp = ctx.enter_context(tc.tile_pool(name="weights", bufs=1))
        sp = ctx.enter_context(tc.tile_pool(name="scratch", bufs=ILV + 1))
        yp = ctx.enter_context(tc.tile_pool(name="ybuf", bufs=ILV + 1))
        hcp = ctx.enter_context(tc.tile_pool(name="hc", bufs=ILV, space="PSUM"))
        ptr = ctx.enter_context(tc.tile_pool(name="ptrans", bufs=ILV,
                                             space="PSUM"))

        def wtile(src, shape, tag, dt=F32):
            if dt == F32:
                t = wp.tile(shape, F32, tag=tag)
                nc.sync.dma_start(t[:], src)
                return t
            st = sp.tile([128, 9 * 128], F32, tag="wstage")
            sv = st[: shape[0], : shape[1]]
            nc.sync.dma_start(sv, src)
            t = wp.tile(shape, dt, tag=tag)
            nc.scalar.copy(t[:], sv)
            return t

        wx_t = wtile(wx_d[:], [2, 128], "wx", F32R)
        wcg_t = wtile(wcg_d[:], [128, 128], "wcg", F32R)
        wl_t = [wtile(wl_d[l], [128, 9 * 128], f"wl{l}", F32R) for l in range(L)]
        wh_t = wtile(wh_d[:], [128, 193], "wh", F32R)
        bias_t = wtile(bias_d[:], [128, 37], "bias", F32)
        Jv_t = wtile(Jv_d[:], [128, 128], "Jv", F32R)
        sm_f = wtile(sm_d[:], [128, 4], "smf", F32)
        sm_t = wp.tile([128, 4], BF16, tag="smb")
        nc.vector.tensor_copy(sm_t[:], sm_f[:])
        bc_f = wtile(bc_d[:], [36, 256], "bcf", F32)
        bc_t = wp.tile([36, 256], BF16, tag="bcb")
        nc.vector.tensor_copy(bc_t[:], bc_f[:])

        bd_t = []
        for r in range(11):
            st = sp.tile([128, 9 * 128], F32, tag="wstage")
            nc.sync.dma_start(st[0:1, 0:128], biasd_d[r:r + 1, :])
            t = wp.tile([1, 128], F32R, tag=f"bd{r}")
            nc.scalar.copy(t[:], st[0:1, 0:128])
            bd_t.append(t)

        eps_st = sp.tile([128, 9 * 128], F32, tag="wstage")
        nc.vector.memset(eps_st[0:1, 0:128], EPS / 2.0)
        eps_t = wp.tile([1, 128], F32R, tag="epsr")
        nc.scalar.copy(eps_t[:], eps_st[0:1, 0:128])
        one_st = sp.tile([128, 9 * 128], F32, tag="wstage")
        nc.vector.memset(one_st[0:1, 0:NT], 1.0)
        ones_t = wp.tile([1, NT], F32R, tag="ones")
        nc.scalar.copy(ones_t[:], one_st[0:1, 0:NT])

        def bcol(i):
            return bias_t[:, i:i + 1]

        def mm(out, lhsT, rhs, start, stop, skip=False):
            # each 512-col chunk lands in its own PSUM bank, which has its
            # own accumulation-group lifecycle -> pass flags through
            ncols = out.shape[-1]
            nch = (ncols + MMC - 1) // MMC
            for c in range(nch):
                sl = slice(c * MMC, min((c + 1) * MMC, ncols))
                nc.tensor.matmul(out[:, sl], lhsT, rhs[:, sl],
                                 start=start, stop=stop,
                                 skip_group_check=skip)

        def layernorm(tiles, vh_of, eps=False):
            """Phase-major LN across tiles: y = hc * rstd (bias inside hc)."""
            for tl in tiles:
                hc, sc = tl["ln_in"], tl["sc"]
                nc.scalar.activation(out=sc[0][:], in_=hc[:],
                                     func=AF.Square, bias=0.0, scale=1.0)
            for tl in tiles:
                vh = vh_of(tl)
                if eps:
                    mm(vh[:], eps_t[:], ones_t[:], True, False)
                mm(vh[:], Jv_t[:], tl["sc"][0][:], not eps, True)
            for tl in tiles:
                vh, tb = vh_of(tl), tl["sc"][1]
                nc.vector.tensor_scalar(
                    tb[:], vh[:].bitcast(I32), 1, -1,
                    op0=ALU.logical_shift_right, op1=ALU.bitwise_xor)
            for tl in tiles:
                tb = tl["sc"][1]
                nc.vector.tensor_scalar(tb[:], tb[:], SEED_ADD, None,
                                        op0=ALU.add)
            for tl in tiles:
                vh, tb, R = vh_of(tl), tl["sc"][1], tl["sc"][2]
                nc.vector._custom_dve(RSQRT_NR2, out=R[:], in0=vh[:],
                                      in1=tb[:].bitcast(F32), s0=1.5)
            for tl in tiles:
                hc, R, y = tl["ln_in"], tl["sc"][2], tl["sc"][3]
                nc.vector.tensor_mul(y[:], hc[:], R[:])

        def emit_embed_group(its):
            tiles = []
            for it in its:
                b0 = it * N
                xs = sp.tile([2, 2, N], F32, tag="xs")
                nc.sync.dma_start(
                    xs[:, 0, :], x_d[b0:b0 + N, 0:2].rearrange("n f -> f n"))
                nc.sync.dma_start(
                    xs[:, 1, :], x_d[b0:b0 + N, 2:4].rearrange("n f -> f n"))
                ta = ptr.tile([128, NT], F32, tag="big")
                hc = hcp.tile([128, NT], F32, tag="hc")
                sq_t = sp.tile([128, NT], F32R, tag="sq")
                tb_t = sp.tile([128, NT], I32, tag="tbits")
                r_t = sp.tile([128, NT], F32, tag="rstd")
                y_t = yp.tile([128, NT], F32R, tag="yln")
                xr = sp.tile([2, 2, N], F32R, tag="xr")
                nc.scalar.copy(xr[:], xs[:])
                tiles.append({"it": it, "ta": ta, "hc": hc,
                              "sc": (sq_t, tb_t, r_t, y_t), "xr": xr})
            for tl in tiles:
                ec, xr = tl["ta"], tl["xr"]
                nc.tensor.matmul(ec[:, 0:N], wx_t[:], xr[:, 0, :],
                                 start=True, stop=bd_zero[0])
                nc.tensor.matmul(ec[:, N:NT], wx_t[:], xr[:, 1, :],
                                 start=True, stop=bd_zero[0])
                if not bd_zero[0]:
                    mm(ec[:], bd_t[0][:], ones_t[:], False, True)
                tl["ln_in"] = ec
            # embed-LN variance borrows the hc bank (overwritten below)
            layernorm(tiles, lambda tl: tl["hc"], eps=True)
            for tl in tiles:
                hc, y_e = tl["hc"], tl["sc"][3]
                mm(hc[:], wcg_t[:], y_e[:], True, False)
                nc.tensor.matmul(hc[:, 0:N], bd_t[1][:], ones_t[:, 0:N],
                                 start=False, stop=True)
                nc.tensor.matmul(hc[:, N:NT], bd_t[2][:], ones_t[:, N:NT],
                                 start=False, stop=True)
                tl["ln_in"] = hc
            return tiles

        def emit_layer_group(l, tiles):
            W = wl_t[l]
            Wq, Wk, Wv = W[:, 0:128], W[:, 128:256], W[:, 256:384]
            CWV, CWh = W[:, 384:512], W[:, 512:640]
            F0, F1 = W[:, 640:768], W[:, 768:896]
            M2a, M2b = W[:, 896:1024], W[:, 1024:1152]

            layernorm(tiles, lambda tl: tl["ta"])
            for tl in tiles:
                y1f = tl["sc"][3][:].bitcast(F32)
                dy = yp.tile([128, N], F32R, tag="dy")
                nc.gpsimd.tensor_tensor(dy[:], y1f[:, 0:N], y1f[:, N:NT],
                                        op=ALU.subtract)
                tl["dy"] = dy
            for tl in tiles:
                y1f = tl["sc"][3][:].bitcast(F32)
                sy = yp.tile([128, N], F32R, tag="sy")
                nc.gpsimd.tensor_tensor(sy[:], y1f[:, 0:N], y1f[:, N:NT],
                                        op=ALU.add)
                tl["sy"] = sy
            for tl in tiles:
                nc.tensor.matmul(tl["ta"][:, 0:N], Wk, tl["dy"][:],
                                 start=True, stop=True)
                nc.tensor.matmul(tl["ta"][:, N:NT], Wv, tl["dy"][:],
                                 start=True, stop=True)
            for tl in tiles:
                kv_sb = sp.tile([128, NT], BF16, tag="kvsb")
                nc.scalar.activation(out=kv_sb[:], in_=tl["ta"][:],
                                     func=AF.Identity, bias=0.0, scale=1.0)
                tl["kv_sb"] = kv_sb
            for tl in tiles:
                mm(tl["ta"][:], Wq, tl["sc"][3][:], True, True)
            for tl in tiles:
                dk = tl["kv_sb"][:, 0:N]
                pr = sp.tile([128, 2, N], BF16, tag="prods")
                dk_b = bass.AP(tensor=dk.tensor, offset=dk.offset,
                               ap=[dk.ap[0], [0, 2], dk.ap[1]])
                nc.vector.scalar_tensor_tensor(
                    pr[:], tl["ta"][:].rearrange("p (q n) -> p q n", q=2),
                    bcol(22 + l), dk_b, op0=ALU.add, op1=ALU.mult)
                tl["pr"] = pr
            for tl in tiles:
                at = tl["ta"]
                nc.tensor.matmul(at[0:4, 0:N], sm_t[:], tl["pr"][:, 0, :],
                                 start=True, stop=True)
                nc.tensor.matmul(at[32:36, 0:N], sm_t[:], tl["pr"][:, 1, :],
                                 start=True, stop=True, tile_position=(0, 32))
            for tl in tiles:
                T8 = sp.tile([36, N], BF16, tag="T8")
                nc.scalar.activation(out=T8[:], in_=tl["ta"][0:36, 0:N],
                                     func=AF.Tanh, bias=0.0, scale=0.5)
                tl["T8"] = T8
            for tl in tiles:
                at = tl["ta"]
                nc.tensor.matmul(at[:, 0:N], bc_t[:, 0:128], tl["T8"][:],
                                 start=True, stop=True)
                nc.tensor.matmul(at[:, N:NT], bc_t[:, 128:256], tl["T8"][:],
                                 start=True, stop=True)
            for tl in tiles:
                dv = tl["kv_sb"][:, N:NT]
                u = yp.tile([128, NT], F32R, tag="u")
                dv_b = bass.AP(tensor=dv.tensor, offset=dv.offset,
                               ap=[dv.ap[0], [0, 2], dv.ap[1]])
                nc.vector.tensor_mul(
                    u[:].rearrange("p (q n) -> p q n", q=2),
                    tl["ta"][:].rearrange("p (q n) -> p q n", q=2), dv_b)
                tl["u"] = u
            for tl in tiles:
                hc = tl["hc"]
                nc.tensor.matmul(hc[:, 0:N], CWV, tl["sy"][:],
                                 start=False, stop=False,
                                 skip_group_check=True)
                nc.tensor.matmul(hc[:, N:NT], CWV, tl["sy"][:],
                                 start=False, stop=False,
                                 skip_group_check=True)
            for tl in tiles:
                mm(tl["hc"][:], CWh, tl["u"][:], False,
                   bd_zero[3 + 2 * l], skip=True)
                if not bd_zero[3 + 2 * l]:
                    mm(tl["hc"][:], bd_t[3 + 2 * l][:], ones_t[:], False,
                       True, skip=True)

            layernorm(tiles, lambda tl: tl["ta"])
            for tl in tiles:
                mm(tl["ta"][:], F0, tl["sc"][3][:], True, True)
            for tl in tiles:
                g0 = yp.tile([128, NT], F32R, tag="g0")
                nc.scalar.activation(out=g0[:], in_=tl["ta"][:], func=AF.Gelu,
                                     bias=bcol(26 + 2 * l), scale=1.0)
                tl["g0"] = g0
            for tl in tiles:
                mm(tl["ta"][:], F1, tl["sc"][3][:], True, True)
            for tl in tiles:
                g1 = yp.tile([128, NT], F32R, tag="g1")
                nc.scalar.activation(out=g1[:], in_=tl["ta"][:], func=AF.Gelu,
                                     bias=bcol(27 + 2 * l), scale=1.0)
                tl["g1"] = g1
            for tl in tiles:
                mm(tl["hc"][:], M2a, tl["g0"][:], False, False, skip=True)
            for tl in tiles:
                mm(tl["hc"][:], M2b, tl["g1"][:], False,
                   bd_zero[4 + 2 * l], skip=True)
                if not bd_zero[4 + 2 * l]:
                    mm(tl["hc"][:], bd_t[4 + 2 * l][:], ones_t[:], False,
                       True, skip=True)
            return tiles

        def emit_head_group(tiles):
            layernorm(tiles, lambda tl: tl["ta"])
            for tl in tiles:
                yf, pp = tl["sc"][3], tl["ta"]
                nc.tensor.matmul(pp[:, 0:N], wh_t[:, 0:128], yf[:, 0:N],
                                 start=True, stop=False)
                nc.tensor.matmul(pp[:, 0:N], wh_t[:, 0:128], yf[:, N:NT],
                                 start=False, stop=True)
            for tl in tiles:
                p1h = sp.tile([128, N], F32R, tag="p1h")
                nc.scalar.activation(out=p1h[:], in_=tl["ta"][:, 0:N],
                                     func=AF.Gelu, bias=bcol(34), scale=1.0)
                tl["p1h"] = p1h
            for tl in tiles:
                nc.tensor.matmul(tl["ta"][0:64, N:NT], wh_t[:, 128:192],
                                 tl["p1h"][:], start=True, stop=True)
            for tl in tiles:
                p2h = sp.tile([64, N], F32R, tag="p2h")
                nc.scalar.activation(out=p2h[:], in_=tl["ta"][0:64, N:NT],
                                     func=AF.Gelu, bias=bias_t[0:64, 35:36],
                                     scale=1.0)
                tl["p2h"] = p2h
            for tl in tiles:
                nc.tensor.matmul(tl["ta"][0:1, 0:N], wh_t[0:64, 192:193],
                                 tl["p2h"][:], start=True, stop=True)
            for tl in tiles:
                th = sp.tile([1, N], F32, tag="th")
                nc.scalar.activation(out=th[:], in_=tl["ta"][0:1, 0:N],
                                     func=AF.Tanh, bias=bias_t[0:1, 36:37],
                                     scale=1.0)
                tl["th"] = th
            for tl in tiles:
                res = sp.tile([1, N], F32, tag="res")
                nc.scalar.mul(res[:], tl["th"][:], 3.0)
                b0 = tl["it"] * N
                nc.sync.dma_start(o_d[0:1, b0:b0 + N], res[:])

        it0 = 0
        while it0 < ntiles:
            g = min(ILV, ntiles - it0)
            tiles = emit_embed_group(list(range(it0, it0 + g)))
            for tl in tiles:
                tl["ln_in"] = tl["hc"]
            for l in range(L):
                tiles = emit_layer_group(l, tiles)
            emit_head_group(tiles)
            it0 += g

    nc.compile()
    return nc


_NC_CACHE = {}


def kernel(**inputs):
    w = _prepare_weights(inputs)
    bd_zero = tuple(bool(np.all(w["biasd"][r] == 0.0)) for r in range(11))
    key = ("nc", bd_zero)
    if key not in _NC_CACHE:
        _NC_CACHE[key] = build_nc(bd_zero=bd_zero)
    nc = _NC_CACHE[key]
    x = np.asarray(inputs["x"], np.float32)
    in_maps = []
    for c in range(NCORES):
        in_maps.append({
            "x": np.ascontiguousarray(x[c * BP:(c + 1) * BP]),
            "wx": w["wx"], "wcg": w["wcg"], "wl": w["wl"],
            "wh": w["wh"], "bias": w["bias"], "biasd": w["biasd"],
        })
    res = run_bass_kernel_spmd(nc, in_maps, core_ids=list(range(NCORES)))
    outs = [res.results[c]["o"].reshape(BP, 1) for c in range(NCORES)]
    return np.concatenate(outs, axis=0).astype(np.float32)


if __name__ == "__main__":
    build_nc(ntiles=2)
    print("build ok")
